# revision 3
# baseline (speedup 1.0000x reference)
"""CWSA (channel-wise self-attention) layer for Trainium2, 8 NeuronCores.

Math (per batch b of 4):
    x_q = W_qk @ x[b]                  # [64, 4096]   (k == q, tied weights)
    x_v = W_v  @ x[b] + b_v            # [64, 4096]
    E   = x_q^T x_q / 8                # [4096, 4096] Gram matrix
    A   = softmax(E, axis=-1)          # rows sum to 1
    out = x_v @ A                      # [64, 4096]
Sharding: 8 cores = 4 batches x 2 halves of the n (row/contraction) axis;
softmax rows stay core-local, each core emits a partial out and the host
sums the two partials per batch.

Design (v2): the exp work is split between the scalar engine (native EXP)
and the vector engine (Schraudolph fast-exp: one tensor_scalar computing
round(E*128*log2e + (127*128 + 128*C)) into int16, whose bit pattern IS
bf16 exp(E) to ~2-4% -- well inside the 2e-2 budget; numpy-validated at
rel-err 0.005 for this offload fraction). Chunk (t,2) of every tile t>=2
goes to DVE; the other 50 chunk-equivalents stay on ACT.

Rowsums never touch gpsimd and cost the vector engine only ~0.6us per
2048 columns: a single in-place tensor_scalar copy (bf16, 4x mode) with
accum_out produces the partial rowsum for free. This removes ~54us of
gpsimd folds and ~35us of vector reduces vs v1 -- both engines were
lighting up the HAM power governor (36us of 50%-throttle windows in the
v1 trace) and starving the PE into low p-states.

Ramp: weights are DMA'd FIRST on both rings (v1 queued them behind the
first 128KB x chunk -- the first projection waited on weights at 12.5us
while x was ready at 11.0us). The first x chunk is split 128+384 cols so
the first projection/fill/exp pipeline starts on a 32KB transfer.

Tail: output banks are copied (scalar/vector alternating) and DMA'd on
three queues (sync/scalar HWDGE + gpsimd SWDGE) as each closes.
"""

import sys

sys.path.insert(0, "/opt/trn_rl_repo")

import numpy as np
import ml_dtypes

import concourse.bass as bass
import concourse.mybir as mybir
import concourse.tile as tile
from concourse import bacc
from concourse.bass import ts, ds

B = 4
C = 256
C4 = 64
N = 4096
NH = N // 2          # n rows per core
NT = 128             # n-tile rows
NTILES = NH // NT    # 16
FACTOR = float(np.sqrt(C4))  # 8.0

BF16 = mybir.dt.bfloat16
F32 = mybir.dt.float32
I16 = mybir.dt.int16
EXP = mybir.ActivationFunctionType.Exp
ADD = mybir.AluOpType.add
MULT = mybir.AluOpType.mult

# Schraudolph constants for bf16 (7-bit mantissa): bits = round(E*SA + SB).
# C = -0.0579 minimizes the softmax-domain L2 error (numpy-calibrated).
SCHR_A = 128.0 / float(np.log(2.0))
SCHR_C = -0.0579
SCHR_B = 127.0 * 128.0 + 128.0 * SCHR_C
# tiles whose chunk 2 is computed on the vector engine
SCHR_TILES = set(range(2, NTILES))


def build_nc() -> bass.Bass:
    nc = bacc.Bacc("TRN2", target_bir_lowering=False, debug=False, num_devices=8)

    x_m = nc.declare_dram_parameter("x_m", [C, N], BF16, isOutput=False)
    wq_t = nc.declare_dram_parameter("wq_t", [C, C4], BF16, isOutput=False)
    wv_t = nc.declare_dram_parameter("wv_t", [C, C4], BF16, isOutput=False)
    bv = nc.declare_dram_parameter("bv", [C4], BF16, isOutput=False)
    out_p = nc.declare_dram_parameter("out_p", [C4, N], BF16, isOutput=True)

    from contextlib import ExitStack

    with tile.TileContext(nc) as tc, ExitStack() as ctx:
        sing = ctx.enter_context(tc.tile_pool(name="sing", bufs=1))
        small = ctx.enter_context(tc.tile_pool(name="small", bufs=6))
        work = ctx.enter_context(tc.tile_pool(name="work", bufs=10))
        e_ps = ctx.enter_context(tc.tile_pool(name="e_ps", bufs=2, space="PSUM"))
        xr_ps = ctx.enter_context(tc.tile_pool(name="xr_ps", bufs=1, space="PSUM"))

        # ---- input loads -------------------------------------------------
        # Weights FIRST on both rings (they gate the first projection), then
        # x column chunks in exp-stream order, first chunk split 128+384.
        xm_sb = sing.tile([128, 2, N], BF16)
        wq_sb = sing.tile([128, 2, C4], BF16)
        wv_sb = sing.tile([128, 2, C4], BF16)
        bv_bc = sing.tile([128, C4], BF16)

        def w_src(w_t):
            ap = w_t[:]
            return bass.AP(
                tensor=ap.tensor,
                offset=0,
                ap=[[C4, 128], [C4 * 128, 2], [1, C4]],
            )

        def x2(ch, a, b):
            return x_m[ts(ch, 128), a:b]

        # c0 rows (channels 0:128) on the HWDGE (sync) ring, c1 rows on the
        # SWDGE (gpsimd) ring; each ring's FIFO delivers low columns first.
        nc.sync.dma_start(out=wq_sb, in_=w_src(wq_t))
        nc.sync.dma_start(out=xm_sb[:, 0, 0:128], in_=x2(0, 0, 128))
        nc.sync.dma_start(out=xm_sb[:, 0, 128:512], in_=x2(0, 128, 512))
        nc.sync.dma_start(out=xm_sb[:, 0, 512:1024], in_=x2(0, 512, 1024))
        nc.sync.dma_start(out=xm_sb[:, 0, 1024:2048], in_=x2(0, 1024, 2048))
        nc.sync.dma_start(out=xm_sb[:, 0, 2048:3072], in_=x2(0, 2048, 3072))
        nc.sync.dma_start(out=xm_sb[:, 0, 3072:4096], in_=x2(0, 3072, 4096))
        nc.gpsimd.dma_start(out=wv_sb, in_=w_src(wv_t))
        bv_ap = bv[:]
        bv_bcast = bass.AP(
            tensor=bv_ap.tensor, offset=bv_ap.offset, ap=[[0, 128]] + list(bv_ap.ap)
        )
        nc.gpsimd.dma_start(out=bv_bc, in_=bv_bcast)
        nc.gpsimd.dma_start(out=xm_sb[:, 1, 0:128], in_=x2(1, 0, 128))
        nc.gpsimd.dma_start(out=xm_sb[:, 1, 128:512], in_=x2(1, 128, 512))
        nc.gpsimd.dma_start(out=xm_sb[:, 1, 512:1024], in_=x2(1, 512, 1024))
        nc.gpsimd.dma_start(out=xm_sb[:, 1, 1024:2048], in_=x2(1, 1024, 2048))
        nc.gpsimd.dma_start(out=xm_sb[:, 1, 2048:3072], in_=x2(1, 2048, 3072))
        nc.gpsimd.dma_start(out=xm_sb[:, 1, 3072:4096], in_=x2(1, 3072, 4096))

        # ---- projections -------------------------------------------------
        # q is stored twice along partitions (0:64 and 64:128) so energy
        # fills can row-slot-pack two K=64 matmuls into the PE array.
        def colpack_proj(dst_ps, rhs0, rhs1):
            return [
                nc.tensor.matmul(dst_ps[0:64, :], wq_sb[:, 0, :], rhs0,
                                 start=True, stop=False, tile_position=(0, 0)),
                nc.tensor.matmul(dst_ps[64:128, :], wq_sb[:, 0, :], rhs0,
                                 start=True, stop=False, tile_position=(0, 64),
                                 skip_group_check=True),
                nc.tensor.matmul(dst_ps[0:64, :], wq_sb[:, 1, :], rhs1,
                                 start=False, stop=True, tile_position=(0, 0)),
                nc.tensor.matmul(dst_ps[64:128, :], wq_sb[:, 1, :], rhs1,
                                 start=False, stop=True, tile_position=(0, 64),
                                 skip_group_check=True),
            ]

        xqt = [sing.tile([128, 1024], BF16, name=f"xq{i}") for i in range(4)]

        def xk(row, t):
            i, off = (t * NT) // 1024, (t * NT) % 1024
            return xqt[i][row:row + 64, off:off + NT]

        def xq(row, col, w):
            i, cc = col // 1024, col % 1024
            return xqt[i][row:row + 64, cc:cc + w]

        def q_proj_cols(col0, w, prio=0, tag=None):
            qp = xr_ps.tile([128, w], F32, tag=tag or f"xr{(col0 // 512) % 4}",
                            name=f"qp{col0}")
            mms = colpack_proj(qp, xm_sb[:, 0, ds(col0, w)],
                               xm_sb[:, 1, ds(col0, w)])
            for m in mms:
                m.ins.bass_priority = prio
            i, cc = col0 // 1024, col0 % 1024
            dst = xqt[i][:, cc:cc + w]
            # all casts on vector at high priority: the q pipeline gates the
            # whole exp stream during the ramp.
            cp = nc.vector.tensor_copy(out=dst, in_=qp)
            cp.ins.bass_priority = -600

        # ---- energy fill plumbing ----------------------------------------
        # opening: tile-0 exps start after only 2x32KB of x has landed
        # (sub-chunks a0=cols 0:128, a1=128:512 use just the first small
        # projection); the 'b' halves and later q projections hide under
        # the opening exps.
        chunk_list = [(0, 0, 'a0'), (0, 0, 'a1'),
                      (1, 0, 'a'), (2, 0, 'a'), (3, 0, 'a'),
                      (0, 0, 'b'), (1, 0, 'b'), (2, 0, 'b'), (3, 0, 'b'),
                      (0, 1, None), (1, 1, None), (2, 1, None), (3, 1, None),
                      (0, 2, None), (0, 3, None), (1, 2, None), (1, 3, None),
                      (2, 2, None), (2, 3, None), (3, 2, None), (3, 3, None)]
        for t in range(4, NTILES):
            chunk_list += [(t, 0, None), (t, 1, None),
                           (t, 2, None), (t, 3, None)]

        def is_dve(t, c, sub):
            return c == 2 and sub is None and t in SCHR_TILES

        def emit_fill(t, c, sub=None, prio=0):
            m0 = 1024 * c
            if sub == 'a0':
                e_t = e_ps.tile([128, 128], F32, tag="e", name=f"e{t}_{c}a0")
                m1 = nc.tensor.matmul(e_t, xk(0, t), xq(0, 0, 128),
                                      start=True, stop=True,
                                      tile_position=(0, 0))
                m1.ins.bass_priority = prio
                return e_t
            if sub == 'a1':
                e_t = e_ps.tile([128, 384], F32, tag="e", name=f"e{t}_{c}a1")
                m1 = nc.tensor.matmul(e_t, xk(0, t), xq(0, 128, 384),
                                      start=True, stop=True,
                                      tile_position=(0, 0))
                m1.ins.bass_priority = prio
                return e_t
            if sub == 'a':
                e_t = e_ps.tile([128, 512], F32, tag="e", name=f"e{t}_{c}a")
                m1 = nc.tensor.matmul(e_t, xk(0, t), xq(0, m0, 512),
                                      start=True, stop=True,
                                      tile_position=(0, 0))
                m1.ins.bass_priority = prio
                return e_t
            if sub == 'b':
                e_t = e_ps.tile([128, 512], F32, tag="e", name=f"e{t}_{c}b")
                m1 = nc.tensor.matmul(e_t, xk(64, t), xq(64, m0 + 512, 512),
                                      start=True, stop=True,
                                      tile_position=(64, 0),
                                      skip_group_check=True)
                m1.ins.bass_priority = prio
                return e_t
            e_t = e_ps.tile([128, 1024], F32, tag="e", name=f"e{t}_{c}")
            m1 = nc.tensor.matmul(e_t[:, 0:512], xk(0, t), xq(0, m0, 512),
                                  start=True, stop=True, tile_position=(0, 0))
            m2 = nc.tensor.matmul(e_t[:, 512:1024], xk(64, t),
                                  xq(64, m0 + 512, 512),
                                  start=True, stop=True, tile_position=(64, 0),
                                  skip_group_check=True)
            m1.ins.bass_priority = prio
            m2.ins.bass_priority = prio
            return e_t

        # prologue: projections and the first fills, interleaved so each
        # fill is emitted as soon as its q columns exist.
        q_proj_cols(0, 128, prio=-3000, tag="xr0")
        etiles = {(0, 0, 'a0'): emit_fill(0, 0, 'a0', prio=-2998)}
        q_proj_cols(128, 384, prio=-2996, tag="xr1")
        etiles[(0, 0, 'a1')] = emit_fill(0, 0, 'a1', prio=-2994)
        q_proj_cols(512, 512, prio=-2990, tag="xr2")
        q_proj_cols(1024, 512, prio=-2970, tag="xr3")
        q_proj_cols(1536, 512, prio=-2960, tag="xr0")
        q_proj_cols(2048, 512, prio=-2930, tag="xr1")
        q_proj_cols(2560, 512, prio=-2920, tag="xr2")
        q_proj_cols(3072, 512, prio=-2910, tag="xr3")
        q_proj_cols(3584, 512, prio=-2900, tag="xr0")

        # per-tile v projections (deprioritized PE gap filler)
        xvt_sb = [
            sing.tile([128, C4], BF16, name=f"xvt{t}") for t in range(NTILES)
        ]
        for t in range(NTILES):
            vp = xr_ps.tile([128, C4], F32, tag=f"xr{t % 4}", name=f"vp{t}")
            half = t // 8
            off = (t % 8) * NT
            mm1 = nc.tensor.matmul(vp, xm_sb[:, 0, ds(half * 1024 + off, NT)],
                                   wv_sb[:, 0, :], start=True, stop=False)
            mm2 = nc.tensor.matmul(vp, xm_sb[:, 1, ds(half * 1024 + off, NT)],
                                   wv_sb[:, 1, :], start=False, stop=True)
            mm1.ins.bass_priority = 500_000 + 2 * t
            mm2.ins.bass_priority = 500_000 + 2 * t + 1
            nc.vector.tensor_add(out=xvt_sb[t], in0=vp, in1=bv_bc)

        # ---- output accumulators (partition-packed: even m-chunk in
        # partitions 0-63, odd in 64-127) -----------------------------------
        xr = [
            xr_ps.tile([128, 512], F32, tag=f"xr{k}", name=f"xr{k}")
            for k in range(4)
        ]

        p_tiles = {}
        xvs_tiles = {}
        rs_tiles = {}

        def emit_accum(t, half):
            # partial rowsum over 2048 columns: in-place bf16 copy at 4x
            # with a free accum_out. Replaces gpsimd folds + vector reduces.
            p = p_tiles[t]
            rs = rs_tiles[t]
            src = p[:, ds(2048 * half, 2048)]
            a = nc.vector.tensor_scalar(out=src, in0=src, scalar1=1.0,
                                        scalar2=0.0, op0=MULT, op1=ADD,
                                        accum_out=rs[:, half:half + 1])
            a.ins.bass_priority = -540

        def do_exp(t, c, sub):
            p = p_tiles[t]
            e_t = etiles.pop((t, c, sub))
            if t not in rs_tiles:
                rs_tiles[t] = small.tile([128, 4], F32, tag="rs4",
                                         name=f"rs_{t}")
            rs = rs_tiles[t]
            last_tile = t == NTILES - 1
            if sub == 'a0':
                nc.scalar.activation(out=p[:, 0:128], in_=e_t, func=EXP)
                return
            if sub == 'a1':
                nc.scalar.activation(out=p[:, 128:512], in_=e_t, func=EXP)
                return
            if sub == 'a':
                nc.scalar.activation(out=p[:, ds(1024 * c, 512)],
                                     in_=e_t, func=EXP)
                return
            if sub == 'b':
                nc.scalar.activation(out=p[:, ds(1024 * c + 512, 512)],
                                     in_=e_t, func=EXP)
            elif is_dve(t, c, sub):
                # Schraudolph fast-exp on the vector engine: int16 bits of
                # bf16 exp(E), written through a bitcast view of p.
                dst = p[:, ds(1024 * c, 1024)].bitcast(I16)
                s = nc.vector.tensor_scalar(out=dst, in0=e_t, scalar1=SCHR_A,
                                            scalar2=SCHR_B, op0=MULT, op1=ADD)
                s.ins.bass_priority = -560
                if last_tile:
                    # tile 15 pair-2 rowsum cannot wait for (15,3): sum the
                    # (15,2) chunk alone now, (15,3) carries its own sum.
                    a = nc.vector.tensor_scalar(
                        out=p[:, ds(1024 * c, 1024)],
                        in0=p[:, ds(1024 * c, 1024)], scalar1=1.0,
                        scalar2=0.0, op0=MULT, op1=ADD,
                        accum_out=rs[:, 1:2])
                    a.ins.bass_priority = -545
            elif last_tile and c == 3:
                # the very last exp carries its own rowsum accumulator so
                # the final normalization starts right after it.
                nc.scalar.activation(out=p[:, ds(1024 * c, 1024)], in_=e_t,
                                     func=EXP, accum_out=rs[:, 2:3])
            else:
                nc.scalar.activation(out=p[:, ds(1024 * c, 1024)], in_=e_t,
                                     func=EXP)

        def rowsum_tile(t):
            rs = rs_tiles.pop(t)
            last_tile = t == NTILES - 1
            w = 3 if last_tile else 2
            rsum = small.tile([128, 1], F32, tag="rs")
            r1 = nc.vector.tensor_reduce(out=rsum, in_=rs[:, 0:w],
                                         axis=mybir.AxisListType.X, op=ADD)
            rr = small.tile([128, 1], F32, tag="rr")
            r2 = nc.vector.reciprocal(out=rr, in_=rsum)
            xvs = small.tile([128, C4], BF16, tag="xvs")
            r3 = nc.vector.tensor_scalar_mul(out=xvs, in0=xvt_sb[t], scalar1=rr)
            # the normalization chain gates AV(t): never let the scheduler
            # slip bulk work ahead of it on the vector queue.
            for r in (r1, r2, r3):
                r.ins.bass_priority = -500
            xvs_tiles[t] = xvs

        def emit_av_bank(t, k):
            # one bank's worth of AV (2 matmuls): emitted at separate
            # stream positions so the in-order PE never sees an AV burst
            # longer than ~1us between energy fills.
            p = p_tiles[t]
            xvs = xvs_tiles[t]
            first = t == 0
            last = t == NTILES - 1
            for j in (2 * k, 2 * k + 1):
                po = (j % 2) * 64
                mm = nc.tensor.matmul(
                    xr[k][po:po + 64, :], xvs,
                    p[:, ds(j * 512, 512)],
                    start=first, stop=last, tile_position=(0, po),
                    skip_group_check=True,
                )
                if not last:
                    mm.ins.bass_priority = 1_000_000 + t * 100 + j * 4

        def emit_av(t):
            for k in range(4):
                emit_av_bank(t, k)
            xvs_tiles.pop(t)

        # ---- the stream --------------------------------------------------
        # AV(t) is emitted one tile late (at (t+1, c)) so in the in-order
        # PE queue ALL of tile t+1's fills statically precede AV(t).
        for i, (t, c, sub) in enumerate(chunk_list):
            if t not in p_tiles:
                p_tiles[t] = work.tile([128, N], BF16, tag="p", name=f"p{t}")
            do_exp(t, c, sub)
            if i + 2 < len(chunk_list):
                nt_, nc_, ns_ = chunk_list[i + 2]
                if (nt_, nc_, ns_) not in etiles:
                    prio = -2950 + i * 5 if i < 9 else 0
                    etiles[(nt_, nc_, ns_)] = emit_fill(nt_, nc_, ns_,
                                                        prio=prio)
            # pair-1 rowsum after (t,1) for opening tiles; for t>=4 it is
            # emitted after the (t,2) Schraudolph op -- the in-order DVE
            # queue must start schr the moment its fill lands, or the
            # 2-slot energy ring stalls the ACT stream behind it.
            # pair-2 after (t,3) except tile 15 (whose chunk 2 sums itself
            # and chunk 3 rides the ACT accum).
            if c == 1 and sub is None and t < 4:
                emit_accum(t, 0)
            if c == 2 and sub is None and t >= 4:
                emit_accum(t, 0)
            if c == 3 and sub is None and t != NTILES - 1:
                emit_accum(t, 1)
            if sub is None and t >= 4 and (t - 1) in xvs_tiles:
                emit_av_bank(t - 1, c)
                if c == 3:
                    xvs_tiles.pop(t - 1)
            if c == 3 and sub is None:
                rowsum_tile(t)
                if t < 4 and t >= 1 and (t - 1) in xvs_tiles:
                    emit_av(t - 1)
                if t == NTILES - 1:
                    emit_av(t)

        # ---- epilogue: per-bank staggered PSUM->SBUF copy + DMA ----------
        # bf16 partials: the host sums the two per-batch partials in fp32.
        out_sb = sing.tile([128, 4, 512], BF16)
        for k in range(4):
            if k % 2 == 0:
                nc.scalar.copy(out=out_sb[:, k, :], in_=xr[k])
            else:
                nc.vector.tensor_copy(out=out_sb[:, k, :], in_=xr[k])
        # three queues so the drain of 512KB finishes ~1.7us after the last
        # bank copy instead of ~2.5us on two.
        qs = [nc.sync, nc.scalar, nc.gpsimd]
        for k in range(4):
            qs[(2 * k) % 3].dma_start(out=out_p[:, ts(2 * k, 512)],
                                      in_=out_sb[0:64, k, :])
            qs[(2 * k + 1) % 3].dma_start(out=out_p[:, ts(2 * k + 1, 512)],
                                          in_=out_sb[64:128, k, :])

    nc.compile()
    return nc


_NC_CACHE = None


def _get_nc():
    global _NC_CACHE
    if _NC_CACHE is None:
        _NC_CACHE = build_nc()
    return _NC_CACHE


def make_in_maps(x, W_qk, W_v, b_v):
    bf = ml_dtypes.bfloat16
    x = np.asarray(x, dtype=np.float32)
    W_qk = np.asarray(W_qk, dtype=np.float32)
    W_v = np.asarray(W_v, dtype=np.float32)
    b_v = np.asarray(b_v, dtype=np.float32)
    xbf = np.ascontiguousarray(x).astype(bf)
    wqt = np.ascontiguousarray((W_qk / np.sqrt(FACTOR)).T).astype(bf)
    wvt = np.ascontiguousarray(W_v.T).astype(bf)
    bvb = np.ascontiguousarray(b_v).astype(bf)
    in_maps = []
    for core in range(8):
        b, h = core // 2, core % 2
        xm = xbf[b] if h == 0 else np.ascontiguousarray(
            np.roll(xbf[b], -NH, axis=1))
        in_maps.append({
            "x_m": xm,
            "wq_t": wqt,
            "wv_t": wvt,
            "bv": bvb,
        })
    return in_maps


def kernel(x, W_qk, W_v, b_v, _trace=False):
    from concourse.bass_utils import run_bass_kernel_spmd

    nc = _get_nc()
    in_maps = make_in_maps(x, W_qk, W_v, b_v)
    res = run_bass_kernel_spmd(nc, in_maps, list(range(8)), trace=_trace)
    if _trace:
        print(f"HW exec time: {res.exec_time_ns} ns")
        print(f"mean exec time: {res.mean_exec_time_ns} ns")
    outs = [np.asarray(res.results[i]["out_p"], dtype=np.float32)
            for i in range(8)]
    out = np.stack([
        outs[2 * b] + np.roll(outs[2 * b + 1], NH, axis=1) for b in range(B)
    ])
    return out.astype(np.float32)


# revision 9
# speedup vs baseline: 1.2246x; 1.2246x over previous
"""CWSA (channel-wise self-attention) layer for Trainium2, 8 NeuronCores.

Math (per batch b of 4):
    x_q = W_qk @ x[b]                  # [64, 4096]   (k == q, tied weights)
    x_v = W_v  @ x[b] + b_v            # [64, 4096]
    E   = x_q^T x_q / 8                # [4096, 4096] Gram matrix
    A   = softmax(E, axis=-1)          # rows sum to 1
    out = x_v @ A                      # [64, 4096]
Sharding: 8 cores = 4 batches x 2 halves of the n (row/contraction) axis;
softmax rows stay core-local, each core emits a partial out and the host
sums the two partials per batch.

Design (v2): the exp work is split between the scalar engine (native EXP)
and the vector engine (Schraudolph fast-exp: one tensor_scalar computing
round(E*128*log2e + (127*128 + 128*C)) into int16, whose bit pattern IS
bf16 exp(E) to ~2-4% -- well inside the 2e-2 budget; numpy-validated at
rel-err 0.005 for this offload fraction). Chunk (t,2) of every tile t>=2
goes to DVE; the other 50 chunk-equivalents stay on ACT.

Rowsums never touch gpsimd and cost the vector engine only ~0.6us per
2048 columns: a single in-place tensor_scalar copy (bf16, 4x mode) with
accum_out produces the partial rowsum for free. This removes ~54us of
gpsimd folds and ~35us of vector reduces vs v1 -- both engines were
lighting up the HAM power governor (36us of 50%-throttle windows in the
v1 trace) and starving the PE into low p-states.

Ramp: weights are DMA'd FIRST on both rings (v1 queued them behind the
first 128KB x chunk -- the first projection waited on weights at 12.5us
while x was ready at 11.0us). The first x chunk is split 128+384 cols so
the first projection/fill/exp pipeline starts on a 32KB transfer.

Tail: output banks are copied (scalar/vector alternating) and DMA'd on
three queues (sync/scalar HWDGE + gpsimd SWDGE) as each closes.
"""

import sys

sys.path.insert(0, "/opt/trn_rl_repo")

import numpy as np
import ml_dtypes

import concourse.bass as bass
import concourse.mybir as mybir
import concourse.tile as tile
from concourse import bacc
from concourse.bass import ts, ds

B = 4
C = 256
C4 = 64
N = 4096
NH = N // 2          # n rows per core
NT = 128             # n-tile rows
NTILES = NH // NT    # 16
FACTOR = float(np.sqrt(C4))  # 8.0

BF16 = mybir.dt.bfloat16
F32 = mybir.dt.float32
I16 = mybir.dt.int16
EXP = mybir.ActivationFunctionType.Exp
ADD = mybir.AluOpType.add
MULT = mybir.AluOpType.mult

# Schraudolph constants for bf16 (7-bit mantissa): bits = round(E*SA + SB).
# C = -0.0579 minimizes the softmax-domain L2 error (numpy-calibrated).
SCHR_A = 128.0 / float(np.log(2.0))
SCHR_C = -0.0579
SCHR_B = 127.0 * 128.0 + 128.0 * SCHR_C
# chunks computed on the vector engine: (t,2) for t=2..14, (t,0) for
# t=8..15 -- 21 of 64, sized so ACT (with per-chunk accum_out rowsums)
# and DVE (schraudolph + hf reduces + fixed work) finish together.
SCHR_CHUNKS = {(t, 2) for t in range(2, 15)} | {(t, 0) for t in range(8, 16)}


def build_nc() -> bass.Bass:
    nc = bacc.Bacc("TRN2", target_bir_lowering=False, debug=False, num_devices=8)

    x_m = nc.declare_dram_parameter("x_m", [C, N], BF16, isOutput=False)
    wq_t = nc.declare_dram_parameter("wq_t", [C, C4], BF16, isOutput=False)
    wv_t = nc.declare_dram_parameter("wv_t", [C, C4], BF16, isOutput=False)
    bv = nc.declare_dram_parameter("bv", [C4], BF16, isOutput=False)
    out_p = nc.declare_dram_parameter("out_p", [C4, N], BF16, isOutput=True)

    from contextlib import ExitStack

    with tile.TileContext(nc) as tc, ExitStack() as ctx:
        sing = ctx.enter_context(tc.tile_pool(name="sing", bufs=1))
        small = ctx.enter_context(tc.tile_pool(name="small", bufs=6))
        # deep ring: gpsimd folds must not WAR-wait on vector's backlog
        hfp = ctx.enter_context(tc.tile_pool(name="hfp", bufs=8))
        work = ctx.enter_context(tc.tile_pool(name="work", bufs=10))
        e_ps = ctx.enter_context(tc.tile_pool(name="e_ps", bufs=2, space="PSUM"))
        xr_ps = ctx.enter_context(tc.tile_pool(name="xr_ps", bufs=1, space="PSUM"))

        # ---- input loads -------------------------------------------------
        # Weights FIRST on both rings (they gate the first projection), then
        # x column chunks in exp-stream order, first chunk split 128+384.
        xm_sb = sing.tile([128, 2, N], BF16)
        wq_sb = sing.tile([128, 2, C4], BF16)
        wv_sb = sing.tile([128, 2, C4], BF16)
        bv_bc = sing.tile([128, C4], BF16)

        def w_src(w_t):
            ap = w_t[:]
            return bass.AP(
                tensor=ap.tensor,
                offset=0,
                ap=[[C4, 128], [C4 * 128, 2], [1, C4]],
            )

        def x2(ch, a, b):
            return x_m[ts(ch, 128), a:b]

        # Every projection needs BOTH channel halves; the SWDGE (gpsimd)
        # ring starts ~1.5us later and runs behind, so the ramp-critical
        # first 512 columns of BOTH halves ride the HWDGE (sync) ring.
        # SWDGE carries the c1-half tail plus the (late-needed) v weights.
        nc.sync.dma_start(out=wq_sb, in_=w_src(wq_t))
        nc.sync.dma_start(out=xm_sb[:, 0, 0:128], in_=x2(0, 0, 128))
        nc.sync.dma_start(out=xm_sb[:, 1, 0:128], in_=x2(1, 0, 128))
        nc.sync.dma_start(out=xm_sb[:, 0, 128:512], in_=x2(0, 128, 512))
        nc.sync.dma_start(out=xm_sb[:, 1, 128:512], in_=x2(1, 128, 512))
        nc.sync.dma_start(out=xm_sb[:, 0, 512:1024], in_=x2(0, 512, 1024))
        nc.sync.dma_start(out=xm_sb[:, 0, 1024:2048], in_=x2(0, 1024, 2048))
        nc.sync.dma_start(out=xm_sb[:, 0, 2048:3072], in_=x2(0, 2048, 3072))
        nc.sync.dma_start(out=xm_sb[:, 0, 3072:4096], in_=x2(0, 3072, 4096))
        nc.gpsimd.dma_start(out=wv_sb, in_=w_src(wv_t))
        bv_ap = bv[:]
        bv_bcast = bass.AP(
            tensor=bv_ap.tensor, offset=bv_ap.offset, ap=[[0, 128]] + list(bv_ap.ap)
        )
        nc.gpsimd.dma_start(out=bv_bc, in_=bv_bcast)
        nc.gpsimd.dma_start(out=xm_sb[:, 1, 512:1024], in_=x2(1, 512, 1024))
        nc.gpsimd.dma_start(out=xm_sb[:, 1, 1024:2048], in_=x2(1, 1024, 2048))
        nc.gpsimd.dma_start(out=xm_sb[:, 1, 2048:3072], in_=x2(1, 2048, 3072))
        nc.gpsimd.dma_start(out=xm_sb[:, 1, 3072:4096], in_=x2(1, 3072, 4096))

        # ---- projections -------------------------------------------------
        # q is stored twice along partitions (0:64 and 64:128) so energy
        # fills can row-slot-pack two K=64 matmuls into the PE array.
        def colpack_proj(dst_ps, rhs0, rhs1):
            return [
                nc.tensor.matmul(dst_ps[0:64, :], wq_sb[:, 0, :], rhs0,
                                 start=True, stop=False, tile_position=(0, 0)),
                nc.tensor.matmul(dst_ps[64:128, :], wq_sb[:, 0, :], rhs0,
                                 start=True, stop=False, tile_position=(0, 64),
                                 skip_group_check=True),
                nc.tensor.matmul(dst_ps[0:64, :], wq_sb[:, 1, :], rhs1,
                                 start=False, stop=True, tile_position=(0, 0)),
                nc.tensor.matmul(dst_ps[64:128, :], wq_sb[:, 1, :], rhs1,
                                 start=False, stop=True, tile_position=(0, 64),
                                 skip_group_check=True),
            ]

        xqt = [sing.tile([128, 1024], BF16, name=f"xq{i}") for i in range(4)]

        def xk(row, t):
            i, off = (t * NT) // 1024, (t * NT) % 1024
            return xqt[i][row:row + 64, off:off + NT]

        def xq(row, col, w):
            i, cc = col // 1024, col % 1024
            return xqt[i][row:row + 64, cc:cc + w]

        def q_proj_cols(col0, w, prio=0, tag=None):
            qp = xr_ps.tile([128, w], F32, tag=tag or f"xr{(col0 // 512) % 4}",
                            name=f"qp{col0}")
            mms = colpack_proj(qp, xm_sb[:, 0, ds(col0, w)],
                               xm_sb[:, 1, ds(col0, w)])
            for m in mms:
                m.ins.bass_priority = prio
            i, cc = col0 // 1024, col0 % 1024
            dst = xqt[i][:, cc:cc + w]
            # all casts on vector at high priority: the q pipeline gates the
            # whole exp stream during the ramp.
            cp = nc.vector.tensor_copy(out=dst, in_=qp)
            cp.ins.bass_priority = -600

        # ---- energy fill plumbing ----------------------------------------
        # opening: tile-0 exps start after only 2x32KB of x has landed
        # (sub-chunks a0=cols 0:128, a1=128:512 use just the first small
        # projection); the 'b' halves and later q projections hide under
        # the opening exps.
        chunk_list = [(0, 0, 'a0'), (0, 0, 'a1'),
                      (1, 0, 'a'), (2, 0, 'a'), (3, 0, 'a'),
                      (0, 0, 'b'), (1, 0, 'b'), (2, 0, 'b'), (3, 0, 'b'),
                      (0, 1, None), (1, 1, None), (2, 1, None), (3, 1, None),
                      (0, 2, None), (0, 3, None), (1, 2, None), (1, 3, None),
                      (2, 2, None), (2, 3, None), (3, 2, None), (3, 3, None)]
        for t in range(4, NTILES):
            chunk_list += [(t, 0, None), (t, 1, None),
                           (t, 2, None), (t, 3, None)]

        def is_dve(t, c, sub):
            return sub is None and (t, c) in SCHR_CHUNKS

        def emit_fill(t, c, sub=None, prio=0):
            m0 = 1024 * c
            if sub == 'a0':
                e_t = e_ps.tile([128, 128], F32, tag="e", name=f"e{t}_{c}a0")
                m1 = nc.tensor.matmul(e_t, xk(0, t), xq(0, 0, 128),
                                      start=True, stop=True,
                                      tile_position=(0, 0))
                m1.ins.bass_priority = prio
                return e_t
            if sub == 'a1':
                e_t = e_ps.tile([128, 384], F32, tag="e", name=f"e{t}_{c}a1")
                m1 = nc.tensor.matmul(e_t, xk(0, t), xq(0, 128, 384),
                                      start=True, stop=True,
                                      tile_position=(0, 0))
                m1.ins.bass_priority = prio
                return e_t
            if sub == 'a':
                e_t = e_ps.tile([128, 512], F32, tag="e", name=f"e{t}_{c}a")
                m1 = nc.tensor.matmul(e_t, xk(0, t), xq(0, m0, 512),
                                      start=True, stop=True,
                                      tile_position=(0, 0))
                m1.ins.bass_priority = prio
                return e_t
            if sub == 'b':
                e_t = e_ps.tile([128, 512], F32, tag="e", name=f"e{t}_{c}b")
                m1 = nc.tensor.matmul(e_t, xk(64, t), xq(64, m0 + 512, 512),
                                      start=True, stop=True,
                                      tile_position=(64, 0),
                                      skip_group_check=True)
                m1.ins.bass_priority = prio
                return e_t
            e_t = e_ps.tile([128, 1024], F32, tag="e", name=f"e{t}_{c}")
            m1 = nc.tensor.matmul(e_t[:, 0:512], xk(0, t), xq(0, m0, 512),
                                  start=True, stop=True, tile_position=(0, 0))
            m2 = nc.tensor.matmul(e_t[:, 512:1024], xk(64, t),
                                  xq(64, m0 + 512, 512),
                                  start=True, stop=True, tile_position=(64, 0),
                                  skip_group_check=True)
            m1.ins.bass_priority = prio
            m2.ins.bass_priority = prio
            return e_t

        # prologue: projections and the first fills, interleaved so each
        # fill is emitted as soon as its q columns exist.
        q_proj_cols(0, 128, prio=-3000, tag="xr0")
        etiles = {(0, 0, 'a0'): emit_fill(0, 0, 'a0', prio=-2998)}
        q_proj_cols(128, 384, prio=-2996, tag="xr1")
        etiles[(0, 0, 'a1')] = emit_fill(0, 0, 'a1', prio=-2994)
        q_proj_cols(512, 512, prio=-2990, tag="xr2")
        q_proj_cols(1024, 512, prio=-2970, tag="xr3")
        q_proj_cols(1536, 512, prio=-2960, tag="xr0")
        q_proj_cols(2048, 512, prio=-2930, tag="xr1")
        q_proj_cols(2560, 512, prio=-2920, tag="xr2")
        q_proj_cols(3072, 512, prio=-2910, tag="xr3")
        q_proj_cols(3584, 512, prio=-2900, tag="xr0")

        # per-tile v projections (deprioritized PE gap filler)
        xvt_sb = [
            sing.tile([128, C4], BF16, name=f"xvt{t}") for t in range(NTILES)
        ]
        for t in range(NTILES):
            vp = xr_ps.tile([128, C4], F32, tag=f"xr{t % 4}", name=f"vp{t}")
            half = t // 8
            off = (t % 8) * NT
            mm1 = nc.tensor.matmul(vp, xm_sb[:, 0, ds(half * 1024 + off, NT)],
                                   wv_sb[:, 0, :], start=True, stop=False)
            mm2 = nc.tensor.matmul(vp, xm_sb[:, 1, ds(half * 1024 + off, NT)],
                                   wv_sb[:, 1, :], start=False, stop=True)
            mm1.ins.bass_priority = 500_000 + 2 * t
            mm2.ins.bass_priority = 500_000 + 2 * t + 1
            nc.vector.tensor_add(out=xvt_sb[t], in0=vp, in1=bv_bc)

        # ---- output accumulators (partition-packed: even m-chunk in
        # partitions 0-63, odd in 64-127) -----------------------------------
        xr = [
            xr_ps.tile([128, 512], F32, tag=f"xr{k}", name=f"xr{k}")
            for k in range(4)
        ]

        p_tiles = {}
        xvs_tiles = {}
        rs_tiles = {}

        # rowsum column per chunk: ACT chunks write theirs via accum_out on
        # the exp itself (282ns/instr); DVE chunks get a gpsimd fold + a
        # 512-wide vector reduce. Opening sub-chunks use the spare columns.
        def rs_col(t, c, sub):
            if sub in ('a0', 'a'):
                return 0
            if sub == 'a1':
                return 4
            if sub == 'b':
                return 5 if t == 0 else 4
            return c

        def rs_width(t):
            return 6 if t == 0 else (5 if t <= 3 else 4)

        def do_exp(t, c, sub):
            p = p_tiles[t]
            e_t = etiles.pop((t, c, sub))
            if t not in rs_tiles:
                rs_tiles[t] = small.tile([128, 6], F32, tag="rs6",
                                         name=f"rs_{t}")
            rs = rs_tiles[t]
            col = rs_col(t, c, sub)
            acc = rs[:, col:col + 1]
            if sub == 'a0':
                nc.scalar.activation(out=p[:, 0:128], in_=e_t, func=EXP,
                                     accum_out=acc)
                return
            if sub == 'a1':
                nc.scalar.activation(out=p[:, 128:512], in_=e_t, func=EXP,
                                     accum_out=acc)
                return
            if sub == 'a':
                nc.scalar.activation(out=p[:, ds(1024 * c, 512)],
                                     in_=e_t, func=EXP, accum_out=acc)
                return
            if sub == 'b':
                nc.scalar.activation(out=p[:, ds(1024 * c + 512, 512)],
                                     in_=e_t, func=EXP, accum_out=acc)
            elif is_dve(t, c, sub):
                # Schraudolph fast-exp on the vector engine: int16 bits of
                # bf16 exp(E), written through a bitcast view of p. Its
                # rowsum: gpsimd folds 1024->512, vector reduces 512.
                dst = p[:, ds(1024 * c, 1024)].bitcast(I16)
                s = nc.vector.tensor_scalar(out=dst, in0=e_t, scalar1=SCHR_A,
                                            scalar2=SCHR_B, op0=MULT, op1=ADD)
                s.ins.bass_priority = -560
                hf = hfp.tile([128, 512], BF16, tag="hf")
                nc.gpsimd.tensor_add(out=hf, in0=p[:, ds(1024 * c, 512)],
                                     in1=p[:, ds(1024 * c + 512, 512)])
                r = nc.vector.tensor_reduce(out=acc, in_=hf,
                                            axis=mybir.AxisListType.X, op=ADD)
                r.ins.bass_priority = -540
            else:
                nc.scalar.activation(out=p[:, ds(1024 * c, 1024)], in_=e_t,
                                     func=EXP, accum_out=acc)

        def rowsum_tile(t):
            rs = rs_tiles.pop(t)
            rsum = small.tile([128, 1], F32, tag="rs")
            r1 = nc.vector.tensor_reduce(out=rsum, in_=rs[:, 0:rs_width(t)],
                                         axis=mybir.AxisListType.X, op=ADD)
            rr = small.tile([128, 1], F32, tag="rr")
            r2 = nc.vector.reciprocal(out=rr, in_=rsum)
            xvs = small.tile([128, C4], BF16, tag="xvs")
            r3 = nc.vector.tensor_scalar_mul(out=xvs, in0=xvt_sb[t], scalar1=rr)
            # the normalization chain gates AV(t): never let the scheduler
            # slip bulk work ahead of it on the vector queue.
            for r in (r1, r2, r3):
                r.ins.bass_priority = -500
            xvs_tiles[t] = xvs

        def emit_av_bank(t, k):
            # one bank's worth of AV (2 matmuls): emitted at separate
            # stream positions so the in-order PE never sees an AV burst
            # longer than ~1us between energy fills.
            p = p_tiles[t]
            xvs = xvs_tiles[t]
            first = t == 0
            last = t == NTILES - 1
            for j in (2 * k, 2 * k + 1):
                po = (j % 2) * 64
                mm = nc.tensor.matmul(
                    xr[k][po:po + 64, :], xvs,
                    p[:, ds(j * 512, 512)],
                    start=first, stop=last, tile_position=(0, po),
                    skip_group_check=True,
                )
                if not last:
                    mm.ins.bass_priority = 1_000_000 + t * 100 + j * 4

        def emit_av(t):
            for k in range(4):
                emit_av_bank(t, k)
            xvs_tiles.pop(t)

        # ---- the stream --------------------------------------------------
        # AV(t) is emitted one tile late (at (t+1, c)) so in the in-order
        # PE queue ALL of tile t+1's fills statically precede AV(t).
        for i, (t, c, sub) in enumerate(chunk_list):
            if t not in p_tiles:
                p_tiles[t] = work.tile([128, N], BF16, tag="p", name=f"p{t}")
            do_exp(t, c, sub)
            if i + 2 < len(chunk_list):
                nt_, nc_, ns_ = chunk_list[i + 2]
                if (nt_, nc_, ns_) not in etiles:
                    prio = -2950 + i * 5 if i < 9 else 0
                    etiles[(nt_, nc_, ns_)] = emit_fill(nt_, nc_, ns_,
                                                        prio=prio)
            if sub is None and t >= 4 and (t - 1) in xvs_tiles:
                emit_av_bank(t - 1, c)
                if c == 3:
                    xvs_tiles.pop(t - 1)
            if c == 3 and sub is None:
                rowsum_tile(t)
                if t < 4 and t >= 1 and (t - 1) in xvs_tiles:
                    emit_av(t - 1)
                if t == NTILES - 1:
                    emit_av(t)

        # ---- epilogue: per-bank staggered PSUM->SBUF copy + DMA ----------
        # bf16 partials: the host sums the two per-batch partials in fp32.
        out_sb = sing.tile([128, 4, 512], BF16)
        for k in range(4):
            if k % 2 == 0:
                nc.scalar.copy(out=out_sb[:, k, :], in_=xr[k])
            else:
                nc.vector.tensor_copy(out=out_sb[:, k, :], in_=xr[k])
        # three queues so the drain of 512KB finishes ~1.7us after the last
        # bank copy instead of ~2.5us on two.
        qs = [nc.sync, nc.scalar, nc.gpsimd]
        for k in range(4):
            qs[(2 * k) % 3].dma_start(out=out_p[:, ts(2 * k, 512)],
                                      in_=out_sb[0:64, k, :])
            qs[(2 * k + 1) % 3].dma_start(out=out_p[:, ts(2 * k + 1, 512)],
                                          in_=out_sb[64:128, k, :])

    nc.compile()
    return nc


_NC_CACHE = None


def _get_nc():
    global _NC_CACHE
    if _NC_CACHE is None:
        _NC_CACHE = build_nc()
    return _NC_CACHE


def make_in_maps(x, W_qk, W_v, b_v):
    bf = ml_dtypes.bfloat16
    x = np.asarray(x, dtype=np.float32)
    W_qk = np.asarray(W_qk, dtype=np.float32)
    W_v = np.asarray(W_v, dtype=np.float32)
    b_v = np.asarray(b_v, dtype=np.float32)
    xbf = np.ascontiguousarray(x).astype(bf)
    wqt = np.ascontiguousarray((W_qk / np.sqrt(FACTOR)).T).astype(bf)
    wvt = np.ascontiguousarray(W_v.T).astype(bf)
    bvb = np.ascontiguousarray(b_v).astype(bf)
    in_maps = []
    for core in range(8):
        b, h = core // 2, core % 2
        xm = xbf[b] if h == 0 else np.ascontiguousarray(
            np.roll(xbf[b], -NH, axis=1))
        in_maps.append({
            "x_m": xm,
            "wq_t": wqt,
            "wv_t": wvt,
            "bv": bvb,
        })
    return in_maps


def kernel(x, W_qk, W_v, b_v, _trace=False):
    from concourse.bass_utils import run_bass_kernel_spmd

    nc = _get_nc()
    in_maps = make_in_maps(x, W_qk, W_v, b_v)
    res = run_bass_kernel_spmd(nc, in_maps, list(range(8)), trace=_trace)
    if _trace:
        print(f"HW exec time: {res.exec_time_ns} ns")
        print(f"mean exec time: {res.mean_exec_time_ns} ns")
    outs = [np.asarray(res.results[i]["out_p"], dtype=np.float32)
            for i in range(8)]
    out = np.stack([
        outs[2 * b] + np.roll(outs[2 * b + 1], NH, axis=1) for b in range(B)
    ])
    return out.astype(np.float32)
